# revision 11
# baseline (speedup 1.0000x reference)
"""Trainium2 Bass kernel for multi-head attention decode (B=16, S=8, H=2048,
16 heads x 128 head_dim, KV cache 4096) sharded over 8 NeuronCores by heads
(tensor parallel, 2 heads/core). o_proj partials are summed on the host
(row-parallel unshard) instead of an on-device AllReduce.

v2: bf16 everywhere on device (KV cache, weights, activations; PSUM accum
stays fp32). Per (head,batch) the K^T slice and the chunk-transposed V slice
are packed host-side into one contiguous [128, 8192] bf16 region so phase 2
needs exactly one 2 MiB DMA per (h,b). Expected rel err vs f32 reference
~5e-3 (CPU-simulated), well under the 2e-2 gate.

Self-contained: hardcodes all shapes/sharding. Host side only reshapes /
casts / packs the sharded inputs and sums the per-core partial outputs.
"""

import numpy as np
import ml_dtypes

import concourse.bass as bass
import concourse.tile as tile
import concourse.mybir as mybir
from concourse import bacc
from concourse.bass_utils import run_bass_kernel_spmd

F32 = mybir.dt.float32
BF16 = mybir.dt.bfloat16
NPBF16 = ml_dtypes.bfloat16

N_CORES = 8
B = 16
S = 8
H = 2048
NH = 16           # total heads
HD = 128          # head dim
KV = 4096         # past kv length
NHL = NH // N_CORES   # heads per core = 2
TOK = B * S       # 128 tokens
NCH = KV // 128   # 32 kv chunks per (h,b)
KT16 = H // 128   # 16 contraction tiles over H
SCALE = 1.0 / float(np.sqrt(HD))

# packed const layout (bf16 columns per partition)
XT_OFF = 0
XT_LEN = KT16 * TOK          # 2048
WQ_OFF = XT_OFF + XT_LEN
W_LEN = KT16 * NHL * HD      # 4096
WK_OFF = WQ_OFF + W_LEN
WV_OFF = WK_OFF + W_LEN
WO_OFF = WV_OFF + W_LEN
WO_LEN = NHL * H             # 4096
PROT_OFF = WO_OFF + WO_LEN
CST_COLS = PROT_OFF + HD     # 18560

_CACHED_NC = None


def _build_nc(repeat=None, ablate=frozenset(), kv_bufs=4, kv_pf=3):
    """repeat=R builds a benchmark variant: phases 1-4 wrapped in a hardware
    For_i loop R times (same NEFF size, loop bound only).
    ablate (perf bisection only; breaks outputs): subset of
      {"dmaonly", "noph134"}."""
    ablate = frozenset(ablate)
    nc = bacc.Bacc(
        "TRN2",
        target_bir_lowering=False,
        debug=False,
        enable_asserts=False,
        num_devices=1,
    )

    kvt = nc.dram_tensor("kvt", [NHL, B, 128, 2 * KV], BF16, kind="ExternalInput")
    cst = nc.dram_tensor("cst", [128, CST_COLS], BF16, kind="ExternalInput")
    cst32 = nc.dram_tensor("cst32", [128, 2 * TOK], F32, kind="ExternalInput")
    out = nc.dram_tensor("out", [TOK, H], F32, kind="ExternalOutput")

    with tile.TileContext(nc) as tc:
        with tc.tile_pool(name="const", bufs=1) as const, \
             tc.tile_pool(name="kv_io", bufs=kv_bufs) as kvp:
            cst_sb = const.tile([128, CST_COLS], BF16)
            cst32_sb = const.tile([128, 2 * TOK], F32)
            # consts on the scalar HWDGE queue; kv stream on the sync queue
            nc.scalar.dma_start(out=cst_sb, in_=cst.ap())
            nc.scalar.dma_start(out=cst32_sb, in_=cst32.ap())

            xt_sb = cst_sb[:, XT_OFF:XT_OFF + XT_LEN].rearrange(
                "p (t n) -> p t n", t=KT16)
            wq_sb = cst_sb[:, WQ_OFF:WQ_OFF + W_LEN].rearrange(
                "p (t m) -> p t m", t=KT16)
            wk_sb = cst_sb[:, WK_OFF:WK_OFF + W_LEN].rearrange(
                "p (t m) -> p t m", t=KT16)
            wv_sb = cst_sb[:, WV_OFF:WV_OFF + W_LEN].rearrange(
                "p (t m) -> p t m", t=KT16)
            wo_sb = cst_sb[:, WO_OFF:WO_OFF + WO_LEN].rearrange(
                "p (t n) -> p t n", t=NHL)
            prot_sb = cst_sb[:, PROT_OFF:PROT_OFF + HD]
            cost_sb = cst32_sb[:, 0:TOK]
            sint_sb = cst32_sb[:, TOK:2 * TOK]

            ones_sb = const.tile([128, 1], BF16)
            nc.vector.memset(ones_sb, 1.0)
            onesrow_sb = const.tile([1, TOK], F32)
            nc.vector.memset(onesrow_sb, 1.0)

            qT = [const.tile([HD, TOK], BF16, name=f"qT{h}") for h in range(NHL)]
            kT = [const.tile([HD, TOK], BF16, name=f"kT{h}") for h in range(NHL)]
            vstage = [const.tile([S, B, HD], BF16, name=f"vstage{h}") for h in range(NHL)]
            # unnormalized attention out (transposed) + per-token softmax denoms
            oTu_sb = [const.tile([HD, TOK], F32, name=f"oTu{h}") for h in range(NHL)]
            rsh_sb = [const.tile([1, TOK], F32, name=f"rsh{h}") for h in range(NHL)]
            oT_sb = [const.tile([HD, TOK], BF16, name=f"oT{h}") for h in range(NHL)]

            if ablate:
                for h in range(NHL):
                    nc.vector.memset(qT[h], 0.01)
                    nc.vector.memset(kT[h], 0.01)
                    nc.vector.memset(vstage[h], 0.01)

            _loop = None
            if repeat is not None:
                _loop = tc.For_i(0, repeat, 1)
                _loop.__enter__()

            # ---- Phase 1: projections + RoPE (all in [hd, tok] layout) ----
            with tc.tile_pool(name="proj_ps", bufs=2, space="PSUM") as pps, \
                 tc.tile_pool(name="proj_tmp", bufs=2) as ptp:
              if not ablate:
                for h in range(NHL):
                    for w_sb, dst in ((wq_sb, qT[h]), (wk_sb, kT[h])):
                        ps = pps.tile([128, 128], F32, tag="projps", name="ps")
                        for t in range(KT16):
                            nc.tensor.matmul(
                                ps,
                                lhsT=w_sb[:, t, h * HD:(h + 1) * HD],
                                rhs=xt_sb[:, t, :],
                                start=(t == 0),
                                stop=(t == KT16 - 1),
                            )
                        raw16 = ptp.tile([128, 128], BF16, tag="raw", name="raw16")
                        nc.vector.tensor_copy(out=raw16, in_=ps)
                        rot_ps = pps.tile([128, 128], F32, tag="projps", name="rot_ps")
                        nc.tensor.matmul(rot_ps, lhsT=prot_sb, rhs=raw16,
                                         start=True, stop=True)
                        tmp = ptp.tile([128, 128], F32, tag="tmp", name="tmp")
                        nc.vector.tensor_mul(out=tmp, in0=ps, in1=cost_sb)
                        tmp2 = ptp.tile([128, 128], F32, tag="tmp2", name="tmp2")
                        nc.vector.tensor_mul(out=tmp2, in0=rot_ps, in1=sint_sb)
                        nc.vector.tensor_add(out=dst, in0=tmp2, in1=tmp)

                # v_new = x @ Wv  -> [tok, 2*128] (natural layout)
                ps_v = pps.tile([128, NHL * HD], F32, tag="projps", name="ps_v")
                for t in range(KT16):
                    nc.tensor.matmul(ps_v, lhsT=xt_sb[:, t, :], rhs=wv_sb[:, t, :],
                                     start=(t == 0), stop=(t == KT16 - 1))
                vnew_sb = ptp.tile([128, NHL * HD], BF16, tag="vnew", name="vnew_sb")
                nc.vector.tensor_copy(out=vnew_sb, in_=ps_v)
                # restage per (head, batch) at partition base 0: [s, b, hd]
                for h in range(NHL):
                    for b in range(B):
                        nc.gpsimd.dma_start(
                            out=vstage[h][:, b, :],
                            in_=vnew_sb[b * S:(b + 1) * S, h * HD:(h + 1) * HD],
                        )

            # ---- Phase 2: attention over the KV cache ----
            # Per (h,b): ONE 2MB packed kv DMA (K^T in cols 0:4096, chunk-
            # transposed V in cols 4096:8192); 32+1 scores matmuls into one
            # PSUM bank; ONE exp; 33 attn@V matmuls + rowsums. Software-
            # pipelined by one (h,b) step so PE stays busy during exp.
            with tc.tile_pool(name="esb", bufs=2) as etp, \
                 tc.tile_pool(name="ps_s", bufs=2, space="PSUM") as psp, \
                 tc.tile_pool(name="ps_o", bufs=2, space="PSUM") as pso, \
                 tc.tile_pool(name="ps_rs", bufs=2, space="PSUM") as psr:
                hb = [(h, b) for h in range(NHL) for b in range(B)]

                kvq = {}   # step -> kv tile
                stage = {}  # pipelined state for step i

                def dma_kv(i):
                    h, b = hb[i]
                    t = kvp.tile([128, 2 * KV], BF16, tag="kv", name="kv_t")
                    nc.sync.dma_start(out=t, in_=kvt.ap()[h, b])
                    kvq[i] = t

                def emit_scores(i):
                    if "dmaonly" in ablate:
                        return
                    h, b = hb[i]
                    qcol = qT[h][:, b * S:(b + 1) * S]
                    kv_t = kvq[i]
                    # cols 0..255: past-kv scores; cols 256..263: new-token
                    s_ps = psp.tile([128, (NCH + 1) * S], F32, tag="sps", name="s_ps")
                    for c in range(NCH):
                        nc.tensor.matmul(
                            s_ps[:, c * S:(c + 1) * S],
                            lhsT=kv_t[:, c * 128:(c + 1) * 128],
                            rhs=qcol,
                            start=True, stop=True)
                    nc.tensor.matmul(
                        s_ps[0:S, NCH * S:(NCH + 1) * S],
                        lhsT=kT[h][:, b * S:(b + 1) * S],
                        rhs=qcol, start=True, stop=True)
                    eT = etp.tile([128, (NCH + 1) * S], BF16, tag="eT", name="eT")
                    nc.scalar.activation(out=eT[:, 0:NCH * S], in_=s_ps[:, 0:NCH * S],
                                         func=mybir.ActivationFunctionType.Exp,
                                         scale=SCALE)
                    nc.scalar.activation(out=eT[0:S, NCH * S:(NCH + 1) * S],
                                         in_=s_ps[0:S, NCH * S:(NCH + 1) * S],
                                         func=mybir.ActivationFunctionType.Exp,
                                         scale=SCALE)
                    stage[i] = eT

                def emit_attnv(i):
                    if "dmaonly" in ablate:
                        kvq.pop(i)
                        return
                    h, b = hb[i]
                    eT = stage.pop(i)
                    kv_t = kvq.pop(i)
                    eTn = eT[0:S, NCH * S:(NCH + 1) * S]
                    # oT2_ps[d, slot, s]: rotating accumulators in SEPARATE
                    # PSUM banks so consecutive matmuls never RMW the same
                    # accumulation address (drain pipelining)
                    NSLOT = 2
                    oT2_ps = pso.tile([HD, NSLOT, 512], F32, tag="ops", name="oT2_ps")
                    for c in range(NCH):
                        nc.tensor.matmul(
                            oT2_ps[:, c % NSLOT, 0:S],
                            lhsT=kv_t[:, KV + c * 128:KV + (c + 1) * 128],
                            rhs=eT[:, c * S:(c + 1) * S],
                            start=(c < NSLOT),
                            stop=(c >= NCH - NSLOT + 1))
                    # new tokens (kv positions 4096..4103) -> slot 0, last
                    nc.tensor.matmul(oT2_ps[:, 0, 0:S], lhsT=vstage[h][:, b, :],
                                     rhs=eTn, start=False, stop=True)
                    # rowsums: ones^T @ eT -> [1, (c s)] partials in one matmul
                    rs_ps = psr.tile([1, (NCH + 1) * S], F32, tag="rsps",
                                     name="rs_ps")
                    nc.tensor.matmul(rs_ps[:, 0:NCH * S], lhsT=ones_sb,
                                     rhs=eT[:, 0:NCH * S],
                                     start=True, stop=False)
                    nc.tensor.matmul(rs_ps[:, NCH * S:(NCH + 1) * S],
                                     lhsT=ones_sb[0:S, :],
                                     rhs=eTn, start=False, stop=True)
                    # evacuate: fold the slots -> unnormalized oT column block
                    nc.vector.reduce_sum(
                        out=oTu_sb[h][:, b * S:(b + 1) * S],
                        in_=oT2_ps[:, :, 0:S].rearrange("p g s -> p s g"),
                        axis=mybir.AxisListType.X)
                    nc.vector.reduce_sum(
                        out=rsh_sb[h][:, b * S:(b + 1) * S],
                        in_=rs_ps.rearrange("p (c s) -> p s c", s=S),
                        axis=mybir.AxisListType.X)

                PF = kv_pf  # kv prefetch depth (< kv_bufs)
                for i in range(PF):
                    dma_kv(i)
                emit_scores(0)
                for i in range(1, len(hb)):
                    if i + PF - 1 < len(hb):
                        dma_kv(i + PF - 1)
                    emit_scores(i)
                    emit_attnv(i - 1)
                emit_attnv(len(hb) - 1)

            # ---- Phase 3: normalize per head: oT = oTu * (1/rs) broadcast ----
            with tc.tile_pool(name="ps_bc", bufs=2, space="PSUM") as pbc, \
                 tc.tile_pool(name="nrm", bufs=2) as nrm:
                for h in range(NHL if not ablate else 0):
                    recip = nrm.tile([1, TOK], F32, tag="recip", name="recip")
                    nc.vector.reciprocal(out=recip, in_=rsh_sb[h])
                    bc_ps = pbc.tile([HD, TOK], F32, tag="bc", name="bc_ps")
                    nc.tensor.matmul(bc_ps, lhsT=onesrow_sb, rhs=recip,
                                     start=True, stop=True)
                    nc.vector.tensor_mul(out=oT_sb[h], in0=oTu_sb[h], in1=bc_ps)

            # ---- Phase 4: o_proj; host sums the 8 per-core partials ----
            with tc.tile_pool(name="ps_y", bufs=2, space="PSUM") as psy, \
                 tc.tile_pool(name="ysb", bufs=2) as yp:
                for nb in range(H // 512 if not ablate else 0):
                    y_ps = psy.tile([TOK, 512], F32, tag="yps", name="y_ps")
                    for h in range(NHL):
                        nc.tensor.matmul(
                            y_ps,
                            lhsT=oT_sb[h],
                            rhs=wo_sb[:, h, nb * 512:(nb + 1) * 512],
                            start=(h == 0), stop=(h == NHL - 1))
                    y_sb = yp.tile([TOK, 512], F32, tag="ysb", name="y_sb")
                    nc.vector.tensor_copy(out=y_sb, in_=y_ps)
                    nc.sync.dma_start(out=out.ap()[:, nb * 512:(nb + 1) * 512],
                                      in_=y_sb)

            if _loop is not None:
                _loop.__exit__(None, None, None)

    nc.compile()
    return nc


def get_nc():
    global _CACHED_NC
    if _CACHED_NC is None:
        _CACHED_NC = _build_nc()
    return _CACHED_NC


def _rope_tables():
    inv_freq = (1.0 / (10000.0 ** (np.arange(0, HD, 2, dtype=np.float32) / HD))).astype(np.float32)
    t = np.arange(S, dtype=np.float32)
    freqs = t[:, None] * inv_freq[None, :]          # [S, HD/2]
    emb = np.concatenate([freqs, freqs], axis=-1)   # [S, HD]
    cos = np.cos(emb).astype(np.float32)            # [S, HD]
    sin = np.sin(emb).astype(np.float32)
    # transposed+tiled over batches: [HD, B*S] with col b*S+s = table row s
    cosT = np.tile(cos.T, (1, B)).astype(np.float32)
    sinT = np.tile(sin.T, (1, B)).astype(np.float32)
    return np.ascontiguousarray(cosT), np.ascontiguousarray(sinT)


def _rot_matrix():
    # rot(q)[d] = -q[d+64] (d<64) ; q[d-64] (d>=64);  rot = P @ q (q as [hd] col)
    P = np.zeros((HD, HD), dtype=np.float32)
    half = HD // 2
    for d in range(half):
        P[d, d + half] = -1.0
        P[d + half, d] = 1.0
    return np.ascontiguousarray(P.T)  # lhsT for out = P @ rhs


def _pack_rows(a, rows_per_tile=128):
    """[T*128, N] -> [128, T, N] partition-contiguous packing."""
    t = a.shape[0] // rows_per_tile
    return np.ascontiguousarray(
        a.reshape(t, rows_per_tile, a.shape[1]).transpose(1, 0, 2))


def make_in_maps(x, Wq, Wk, Wv, Wo, past_k, past_v):
    xt = x.reshape(TOK, H).T                                  # [H, TOK]
    cosT, sinT = _rope_tables()
    cst32 = np.ascontiguousarray(np.concatenate([cosT, sinT], axis=1))
    prot = _rot_matrix()

    pk16 = past_k.astype(NPBF16)      # [B, NH, KV, HD]
    pv16 = past_v.astype(NPBF16)

    in_maps = []
    for c in range(N_CORES):
        h0 = c * NHL
        cols = slice(h0 * HD, (h0 + NHL) * HD)
        # K^T: [NHL, B, HD, KV]
        kp = pk16[:, h0:h0 + NHL].transpose(1, 0, 3, 2)
        # V chunk-transposed: [NHL, B, 128, NCH*HD]
        vp = (pv16[:, h0:h0 + NHL]
              .reshape(B, NHL, NCH, 128, HD)
              .transpose(1, 0, 3, 2, 4)
              .reshape(NHL, B, 128, KV))
        kvt = np.concatenate([kp, vp], axis=-1)               # [NHL,B,128,8192]
        cst = np.concatenate([
            _pack_rows(xt).reshape(128, -1),
            _pack_rows(Wq[:, cols]).reshape(128, -1),
            _pack_rows(Wk[:, cols]).reshape(128, -1),
            _pack_rows(Wv[:, cols]).reshape(128, -1),
            _pack_rows(Wo[cols, :]).reshape(128, -1),
            prot,
        ], axis=1).astype(NPBF16)
        in_maps.append({
            "kvt": np.ascontiguousarray(kvt),
            "cst": np.ascontiguousarray(cst),
            "cst32": cst32,
        })
    return in_maps


def kernel(x, Wq, Wk, Wv, Wo, past_k, past_v):
    x = np.asarray(x, dtype=np.float32)
    Wq = np.asarray(Wq, dtype=np.float32)
    Wk = np.asarray(Wk, dtype=np.float32)
    Wv = np.asarray(Wv, dtype=np.float32)
    Wo = np.asarray(Wo, dtype=np.float32)
    past_k = np.asarray(past_k, dtype=np.float32)
    past_v = np.asarray(past_v, dtype=np.float32)

    nc = get_nc()
    in_maps = make_in_maps(x, Wq, Wk, Wv, Wo, past_k, past_v)
    res = run_bass_kernel_spmd(nc, in_maps, core_ids=list(range(N_CORES)))
    y = np.zeros((TOK, H), dtype=np.float32)
    for c in range(N_CORES):
        y += np.asarray(res.results[c]["out"], dtype=np.float32)
    return y.reshape(B, S, H)


# revision 25
# speedup vs baseline: 4.5572x; 4.5572x over previous
"""Trainium2 Bass kernel for multi-head attention decode (B=16, S=8, H=2048,
16 heads x 128 head_dim, KV cache 4096) sharded over 8 NeuronCores by heads
(tensor parallel, 2 heads/core). o_proj partials are summed on the host
(row-parallel unshard) instead of an on-device AllReduce.

v2: fp16 everywhere on device (KV cache, weights, activations; PSUM accum
stays fp32). Per (head,batch) the K^T slice and the chunk-transposed V slice
are packed host-side into one contiguous [128, 8192] fp16 region so phase 2
needs exactly one 2 MiB DMA per (h,b). Expected rel err vs f32 reference
~7e-4 (CPU-simulated), well under the 2e-2 gate.

Self-contained: hardcodes all shapes/sharding. Host side only reshapes /
casts / packs the sharded inputs and sums the per-core partial outputs.
"""

import numpy as np
import ml_dtypes

import concourse.bass as bass
import concourse.tile as tile
import concourse.mybir as mybir
from concourse import bacc
from concourse.bass_utils import run_bass_kernel_spmd

F32 = mybir.dt.float32
HDT = mybir.dt.float16
NPHDT = np.float16
F8 = mybir.dt.float8e3          # e3m4: 4 mantissa bits, max 15.5
NPF8 = ml_dtypes.float8_e3m4
V8 = False   # V cache in e3m4 (halves V DMA; rel err ~1.3e-2 vs 2e-2 gate)

N_CORES = 8
B = 16
S = 8
H = 2048
NH = 16           # total heads
HD = 128          # head dim
KV = 4096         # past kv length
NHL = NH // N_CORES   # heads per core = 2
TOK = B * S       # 128 tokens
NCH = KV // 128   # 32 kv chunks per (h,b)
KT16 = H // 128   # 16 contraction tiles over H
SCALE = 1.0 / float(np.sqrt(HD))

# packed const layout (bf16 columns per partition)
XT_OFF = 0
XT_LEN = KT16 * TOK          # 2048
WQ_OFF = XT_OFF + XT_LEN
W_LEN = KT16 * NHL * HD      # 4096
WK_OFF = WQ_OFF + W_LEN
WV_OFF = WK_OFF + W_LEN
WO_OFF = WV_OFF + W_LEN
WO_LEN = NHL * H             # 4096
PROT_OFF = WO_OFF + WO_LEN
CST_COLS = PROT_OFF + HD     # 18560

_CACHED_NC = None


def _build_nc(repeat=None, ablate=frozenset(), kv_bufs=6, kv_pf=5, v8=None):
    """repeat=R builds a benchmark variant: phases 1-4 wrapped in a hardware
    For_i loop R times (same NEFF size, loop bound only).
    ablate (perf bisection only; breaks outputs): subset of
      {"dmaonly", "noph134"}."""
    ablate = frozenset(ablate)
    if v8 is None:
        v8 = V8
    VDT = F8 if v8 else HDT
    nc = bacc.Bacc(
        "TRN2",
        target_bir_lowering=False,
        debug=False,
        enable_asserts=False,
        num_devices=1,
    )

    if v8:
        kvt = nc.dram_tensor("kvt", [NHL, B, 128, KV], HDT, kind="ExternalInput")
        vt8 = nc.dram_tensor("vt8", [NHL, B, 128, KV], F8, kind="ExternalInput")
    else:
        kvt = nc.dram_tensor("kvt", [NHL, B, 128, 2 * KV], HDT, kind="ExternalInput")
    cst = nc.dram_tensor("cst", [128, CST_COLS], HDT, kind="ExternalInput")
    cst32 = nc.dram_tensor("cst32", [128, 2 * TOK], F32, kind="ExternalInput")
    out = nc.dram_tensor("out", [TOK, H], F32, kind="ExternalOutput")

    with tile.TileContext(nc) as tc:
        with tc.tile_pool(name="const", bufs=1) as const, \
             tc.tile_pool(name="kv_io", bufs=kv_bufs) as kvp:
            cst_sb = const.tile([128, CST_COLS], HDT)
            cst32_sb = const.tile([128, 2 * TOK], F32)
            # consts on the scalar HWDGE queue; kv stream on the sync queue
            nc.scalar.dma_start(out=cst_sb, in_=cst.ap())
            nc.scalar.dma_start(out=cst32_sb, in_=cst32.ap())

            xt_sb = cst_sb[:, XT_OFF:XT_OFF + XT_LEN].rearrange(
                "p (t n) -> p t n", t=KT16)
            wq_sb = cst_sb[:, WQ_OFF:WQ_OFF + W_LEN].rearrange(
                "p (t m) -> p t m", t=KT16)
            wk_sb = cst_sb[:, WK_OFF:WK_OFF + W_LEN].rearrange(
                "p (t m) -> p t m", t=KT16)
            wv_sb = cst_sb[:, WV_OFF:WV_OFF + W_LEN].rearrange(
                "p (t m) -> p t m", t=KT16)
            wo_sb = cst_sb[:, WO_OFF:WO_OFF + WO_LEN].rearrange(
                "p (t n) -> p t n", t=NHL)
            prot_sb = cst_sb[:, PROT_OFF:PROT_OFF + HD]
            cost_sb = cst32_sb[:, 0:TOK]
            sint_sb = cst32_sb[:, TOK:2 * TOK]

            ones_sb = const.tile([128, 1], HDT)
            nc.vector.memset(ones_sb, 1.0)
            onesrow_sb = const.tile([1, TOK], F32)
            nc.vector.memset(onesrow_sb, 1.0)

            qT = [const.tile([HD, TOK], HDT, name=f"qT{h}") for h in range(NHL)]
            kT = [const.tile([HD, TOK], HDT, name=f"kT{h}") for h in range(NHL)]
            vstage = [const.tile([S, B, HD], VDT, name=f"vstage{h}") for h in range(NHL)]
            # unnormalized attention out (transposed) + per-token softmax denoms
            oTu_sb = [const.tile([HD, TOK], F32, name=f"oTu{h}") for h in range(NHL)]
            rsh_sb = [const.tile([1, TOK], F32, name=f"rsh{h}") for h in range(NHL)]
            oT_sb = [const.tile([HD, TOK], HDT, name=f"oT{h}") for h in range(NHL)]

            if ablate:
                for h in range(NHL):
                    nc.vector.memset(qT[h], 0.01)
                    nc.vector.memset(kT[h], 0.01)
                    nc.vector.memset(vstage[h], 0.01)

            _loop = None
            if repeat is not None:
                _loop = tc.For_i(0, repeat, 1)
                _loop.__enter__()

            # ---- Phase 1: projections + RoPE (all in [hd, tok] layout) ----
            with tc.tile_pool(name="proj_ps", bufs=2, space="PSUM") as pps, \
                 tc.tile_pool(name="proj_tmp", bufs=2) as ptp:
              if not ablate:
                for h in range(NHL):
                    for w_sb, dst in ((wq_sb, qT[h]), (wk_sb, kT[h])):
                        ps = pps.tile([128, 128], F32, tag="projps", name="ps")
                        for t in range(KT16):
                            nc.tensor.matmul(
                                ps,
                                lhsT=w_sb[:, t, h * HD:(h + 1) * HD],
                                rhs=xt_sb[:, t, :],
                                start=(t == 0),
                                stop=(t == KT16 - 1),
                            )
                        raw16 = ptp.tile([128, 128], HDT, tag="raw", name="raw16")
                        nc.vector.tensor_copy(out=raw16, in_=ps)
                        rot_ps = pps.tile([128, 128], F32, tag="projps", name="rot_ps")
                        nc.tensor.matmul(rot_ps, lhsT=prot_sb, rhs=raw16,
                                         start=True, stop=True)
                        tmp = ptp.tile([128, 128], F32, tag="tmp", name="tmp")
                        nc.vector.tensor_mul(out=tmp, in0=ps, in1=cost_sb)
                        tmp2 = ptp.tile([128, 128], F32, tag="tmp2", name="tmp2")
                        nc.vector.tensor_mul(out=tmp2, in0=rot_ps, in1=sint_sb)
                        nc.vector.tensor_add(out=dst, in0=tmp2, in1=tmp)

                # v_new = x @ Wv  -> [tok, 2*128] (natural layout)
                ps_v = pps.tile([128, NHL * HD], F32, tag="projps", name="ps_v")
                for t in range(KT16):
                    nc.tensor.matmul(ps_v, lhsT=xt_sb[:, t, :], rhs=wv_sb[:, t, :],
                                     start=(t == 0), stop=(t == KT16 - 1))
                vnew_sb = ptp.tile([128, NHL * HD], VDT, tag="vnew", name="vnew_sb")
                nc.vector.tensor_copy(out=vnew_sb, in_=ps_v)
                # restage per (head, batch) at partition base 0: [s, b, hd]
                for h in range(NHL):
                    for b in range(B):
                        nc.gpsimd.dma_start(
                            out=vstage[h][:, b, :],
                            in_=vnew_sb[b * S:(b + 1) * S, h * HD:(h + 1) * HD],
                        )

            # ---- Phase 2: attention over the KV cache ----
            # Per (h,b): ONE 2MB packed kv DMA (K^T in cols 0:4096, chunk-
            # transposed V in cols 4096:8192); 32+1 scores matmuls into one
            # PSUM bank; ONE exp; 33 attn@V matmuls + rowsums. Software-
            # pipelined by one (h,b) step so PE stays busy during exp.
            with tc.tile_pool(name="esb", bufs=2) as etp, \
                 tc.tile_pool(name="ps_s", bufs=2, space="PSUM") as psp, \
                 tc.tile_pool(name="ps_o", bufs=2, space="PSUM") as pso, \
                 tc.tile_pool(name="ps_rs", bufs=2, space="PSUM") as psr:
                hb = [(h, b) for h in range(NHL) for b in range(B)]

                kvq = {}   # step -> kv tile
                stage = {}  # pipelined state for step i

                def dma_kv(i):
                    h, b = hb[i]
                    if v8:
                        tk = kvp.tile([128, KV], HDT, tag="kv", name="kv_t")
                        tv = kvp.tile([128, KV], F8, tag="v8", name="v8_t")
                        nc.sync.dma_start(out=tk, in_=kvt.ap()[h, b])
                        nc.sync.dma_start(out=tv, in_=vt8.ap()[h, b])
                        kvq[i] = (tk, tv)
                    else:
                        t = kvp.tile([128, 2 * KV], HDT, tag="kv", name="kv_t")
                        # split K/V halves: scores only waits on the K half
                        nc.sync.dma_start(out=t[:, 0:KV],
                                          in_=kvt.ap()[h, b, :, 0:KV])
                        nc.sync.dma_start(out=t[:, KV:2 * KV],
                                          in_=kvt.ap()[h, b, :, KV:2 * KV])
                        kvq[i] = (t, None)

                def emit_scores(i):
                    if "dmaonly" in ablate:
                        return
                    h, b = hb[i]
                    qcol = qT[h][:, b * S:(b + 1) * S]
                    kv_t, _ = kvq[i]
                    # cols 0..255: past-kv scores; cols 256..263: new-token
                    s_ps = psp.tile([128, (NCH + 1) * S], F32, tag="sps", name="s_ps")
                    for c in range(NCH):
                        nc.tensor.matmul(
                            s_ps[:, c * S:(c + 1) * S],
                            lhsT=kv_t[:, c * 128:(c + 1) * 128],
                            rhs=qcol,
                            start=True, stop=True)
                    nc.tensor.matmul(
                        s_ps[0:S, NCH * S:(NCH + 1) * S],
                        lhsT=kT[h][:, b * S:(b + 1) * S],
                        rhs=qcol, start=True, stop=True)
                    eT = etp.tile([128, (NCH + 1) * S], HDT, tag="eT", name="eT")
                    nc.scalar.activation(out=eT[:, 0:NCH * S], in_=s_ps[:, 0:NCH * S],
                                         func=mybir.ActivationFunctionType.Exp,
                                         scale=SCALE)
                    nc.scalar.activation(out=eT[0:S, NCH * S:(NCH + 1) * S],
                                         in_=s_ps[0:S, NCH * S:(NCH + 1) * S],
                                         func=mybir.ActivationFunctionType.Exp,
                                         scale=SCALE)
                    stage[i] = eT

                def emit_attnv(i):
                    if "dmaonly" in ablate:
                        kvq.pop(i)
                        return
                    h, b = hb[i]
                    eT = stage.pop(i)
                    kv_t, v_t = kvq.pop(i)
                    eTn = eT[0:S, NCH * S:(NCH + 1) * S]
                    # oT2_ps[d, slot, s]: rotating accumulators in SEPARATE
                    # PSUM banks so consecutive matmuls never RMW the same
                    # accumulation address (drain pipelining)
                    NSLOT = 2
                    oT2_ps = pso.tile([HD, NSLOT, 512], F32, tag="ops", name="oT2_ps")
                    for c in range(NCH):
                        vch = (v_t[:, c * 128:(c + 1) * 128] if v8 else
                               kv_t[:, KV + c * 128:KV + (c + 1) * 128])
                        nc.tensor.matmul(
                            oT2_ps[:, c % NSLOT, 0:S],
                            lhsT=vch,
                            rhs=eT[:, c * S:(c + 1) * S],
                            start=(c < NSLOT),
                            stop=(c >= NCH - NSLOT + 1))
                    # new tokens (kv positions 4096..4103) -> slot 0, last
                    nc.tensor.matmul(oT2_ps[:, 0, 0:S], lhsT=vstage[h][:, b, :],
                                     rhs=eTn, start=False, stop=True)
                    # rowsums: ones^T @ eT -> [1, (c s)] partials in one matmul
                    rs_ps = psr.tile([1, (NCH + 1) * S], F32, tag="rsps",
                                     name="rs_ps")
                    nc.tensor.matmul(rs_ps[:, 0:NCH * S], lhsT=ones_sb,
                                     rhs=eT[:, 0:NCH * S],
                                     start=True, stop=False)
                    nc.tensor.matmul(rs_ps[:, NCH * S:(NCH + 1) * S],
                                     lhsT=ones_sb[0:S, :],
                                     rhs=eTn, start=False, stop=True)
                    # evacuate: fold the slots -> unnormalized oT column block
                    nc.vector.reduce_sum(
                        out=oTu_sb[h][:, b * S:(b + 1) * S],
                        in_=oT2_ps[:, :, 0:S].rearrange("p g s -> p s g"),
                        axis=mybir.AxisListType.X)
                    nc.vector.reduce_sum(
                        out=rsh_sb[h][:, b * S:(b + 1) * S],
                        in_=rs_ps.rearrange("p (c s) -> p s c", s=S),
                        axis=mybir.AxisListType.X)

                PF = kv_pf  # kv prefetch depth (< kv_bufs)
                for i in range(PF):
                    dma_kv(i)
                emit_scores(0)
                for i in range(1, len(hb)):
                    if i + PF - 1 < len(hb):
                        dma_kv(i + PF - 1)
                    emit_scores(i)
                    emit_attnv(i - 1)
                emit_attnv(len(hb) - 1)

            # ---- Phase 3: normalize per head: oT = oTu * (1/rs) broadcast ----
            with tc.tile_pool(name="ps_bc", bufs=2, space="PSUM") as pbc, \
                 tc.tile_pool(name="nrm", bufs=2) as nrm:
                for h in range(NHL if not ablate else 0):
                    recip = nrm.tile([1, TOK], F32, tag="recip", name="recip")
                    nc.vector.reciprocal(out=recip, in_=rsh_sb[h])
                    bc_ps = pbc.tile([HD, TOK], F32, tag="bc", name="bc_ps")
                    nc.tensor.matmul(bc_ps, lhsT=onesrow_sb, rhs=recip,
                                     start=True, stop=True)
                    nc.vector.tensor_mul(out=oT_sb[h], in0=oTu_sb[h], in1=bc_ps)

            # ---- Phase 4: o_proj; host sums the 8 per-core partials ----
            with tc.tile_pool(name="ps_y", bufs=2, space="PSUM") as psy, \
                 tc.tile_pool(name="ysb", bufs=2) as yp:
                for nb in range(H // 512 if not ablate else 0):
                    y_ps = psy.tile([TOK, 512], F32, tag="yps", name="y_ps")
                    for h in range(NHL):
                        nc.tensor.matmul(
                            y_ps,
                            lhsT=oT_sb[h],
                            rhs=wo_sb[:, h, nb * 512:(nb + 1) * 512],
                            start=(h == 0), stop=(h == NHL - 1))
                    y_sb = yp.tile([TOK, 512], F32, tag="ysb", name="y_sb")
                    nc.vector.tensor_copy(out=y_sb, in_=y_ps)
                    nc.scalar.dma_start(out=out.ap()[:, nb * 512:(nb + 1) * 512],
                                        in_=y_sb)

            if _loop is not None:
                _loop.__exit__(None, None, None)

    nc.compile()
    return nc


def get_nc():
    global _CACHED_NC
    if _CACHED_NC is None:
        _CACHED_NC = _build_nc()
    return _CACHED_NC


def _rope_tables():
    inv_freq = (1.0 / (10000.0 ** (np.arange(0, HD, 2, dtype=np.float32) / HD))).astype(np.float32)
    t = np.arange(S, dtype=np.float32)
    freqs = t[:, None] * inv_freq[None, :]          # [S, HD/2]
    emb = np.concatenate([freqs, freqs], axis=-1)   # [S, HD]
    cos = np.cos(emb).astype(np.float32)            # [S, HD]
    sin = np.sin(emb).astype(np.float32)
    # transposed+tiled over batches: [HD, B*S] with col b*S+s = table row s
    cosT = np.tile(cos.T, (1, B)).astype(np.float32)
    sinT = np.tile(sin.T, (1, B)).astype(np.float32)
    return np.ascontiguousarray(cosT), np.ascontiguousarray(sinT)


def _rot_matrix():
    # rot(q)[d] = -q[d+64] (d<64) ; q[d-64] (d>=64);  rot = P @ q (q as [hd] col)
    P = np.zeros((HD, HD), dtype=np.float32)
    half = HD // 2
    for d in range(half):
        P[d, d + half] = -1.0
        P[d + half, d] = 1.0
    return np.ascontiguousarray(P.T)  # lhsT for out = P @ rhs


def _pack_rows(a, rows_per_tile=128):
    """[T*128, N] -> [128, T, N] partition-contiguous packing."""
    t = a.shape[0] // rows_per_tile
    return np.ascontiguousarray(
        a.reshape(t, rows_per_tile, a.shape[1]).transpose(1, 0, 2))


def make_in_maps(x, Wq, Wk, Wv, Wo, past_k, past_v, v8=None):
    if v8 is None:
        v8 = V8
    xt = x.reshape(TOK, H).T                                  # [H, TOK]
    cosT, sinT = _rope_tables()
    cst32 = np.ascontiguousarray(np.concatenate([cosT, sinT], axis=1))
    prot = _rot_matrix()

    pk16 = past_k.astype(NPHDT)      # [B, NH, KV, HD]
    pv16 = past_v.astype(NPF8 if v8 else NPHDT)

    in_maps = []
    for c in range(N_CORES):
        h0 = c * NHL
        cols = slice(h0 * HD, (h0 + NHL) * HD)
        # K^T: [NHL, B, HD, KV]
        kp = pk16[:, h0:h0 + NHL].transpose(1, 0, 3, 2)
        # V chunk-transposed: [NHL, B, 128, NCH*HD]
        vp = (pv16[:, h0:h0 + NHL]
              .reshape(B, NHL, NCH, 128, HD)
              .transpose(1, 0, 3, 2, 4)
              .reshape(NHL, B, 128, KV))
        cst = np.concatenate([
            _pack_rows(xt).reshape(128, -1),
            _pack_rows(Wq[:, cols]).reshape(128, -1),
            _pack_rows(Wk[:, cols]).reshape(128, -1),
            _pack_rows(Wv[:, cols]).reshape(128, -1),
            _pack_rows(Wo[cols, :]).reshape(128, -1),
            prot,
        ], axis=1).astype(NPHDT)
        m = {
            "cst": np.ascontiguousarray(cst),
            "cst32": cst32,
        }
        if v8:
            m["kvt"] = np.ascontiguousarray(kp)
            m["vt8"] = np.ascontiguousarray(vp)
        else:
            m["kvt"] = np.ascontiguousarray(
                np.concatenate([kp, vp], axis=-1))        # [NHL,B,128,8192]
        in_maps.append(m)
    return in_maps


def kernel(x, Wq, Wk, Wv, Wo, past_k, past_v):
    x = np.asarray(x, dtype=np.float32)
    Wq = np.asarray(Wq, dtype=np.float32)
    Wk = np.asarray(Wk, dtype=np.float32)
    Wv = np.asarray(Wv, dtype=np.float32)
    Wo = np.asarray(Wo, dtype=np.float32)
    past_k = np.asarray(past_k, dtype=np.float32)
    past_v = np.asarray(past_v, dtype=np.float32)

    nc = get_nc()
    in_maps = make_in_maps(x, Wq, Wk, Wv, Wo, past_k, past_v)
    res = run_bass_kernel_spmd(nc, in_maps, core_ids=list(range(N_CORES)))
    y = np.zeros((TOK, H), dtype=np.float32)
    for c in range(N_CORES):
        y += np.asarray(res.results[c]["out"], dtype=np.float32)
    return y.reshape(B, S, H)
